# revision 13
# baseline (speedup 1.0000x reference)
"""Trainium2 Bass kernel for nn_A2Module (area attention + LayerNorm).

Sharding: data-parallel over batch B=8 across the 8 NeuronCores (one image
per core, weights replicated, no collectives).

Per-core math (shapes per area a of 4, L=1024 tokens, C=256, 8 heads x 32):
  xaT[c,l]   : area tokens, channels on partitions
  qkT[d,l]   = W_in[a][qk] @ xaT  (+b)      d on partitions -> heads are
                                            32-partition slices
  v[m,d]     = xaT.T @ W_in[a][v].T (+b)    natural layout, m on partitions
  S^T[m,l]   = kT_h.T @ qT_h                K=32 row-tiled, 4 heads packed
  P^T        = exp(S^T/sqrt(dh))            ScalarE, PSUM->SBUF bf16
  o^T[d,l]   = sum_m v[m,d] P^T[m,l]        M=32 col-tiled, 4 heads packed
  den[l]     = sum_m P^T[m,l]               ones-matmul, result broadcast
  o^T       /= den                           (softmax normalization)
  y^T        = W_out[a] @ o^T + b + xaT     residual via identity matmul
  out        = LN(y) over c                 stats via ones-matmuls (broadcast
                                            across partitions), applied on DVE
"""

import sys

for _p in ("/opt/trn_rl_repo",):
    if _p not in sys.path:
        sys.path.insert(0, _p)

import numpy as np

import concourse.bacc as bacc
import concourse.bass as bass
import concourse.mybir as mybir
import concourse.tile as tile
from concourse.bass_utils import run_bass_kernel_spmd
from concourse.masks import make_identity

F32 = mybir.dt.float32
BF16 = mybir.dt.bfloat16
AluOp = mybir.AluOpType
ActFn = mybir.ActivationFunctionType

B = 8
C = 256
HDIM = 64
WDIM = 64
A = 4
NH = 8
DH = 32
L = 1024
EPS = 1e-5
SCALE = float(DH) ** -0.5


def _build_body(tc, nc, x, W_in, b_in, W_out, b_out, gamma, beta, out_ext):
    consts = tc.alloc_tile_pool(name="consts", bufs=1)

    ident = consts.tile([128, 128], BF16, name="ident")
    make_identity(nc, ident)
    ones32 = consts.tile([128, 32], BF16, name="ones32")
    nc.vector.memset(ones32, 1.0)
    onesrow = consts.tile([1, 128], BF16, name="onesrow")
    nc.vector.memset(onesrow, 1.0)
    negmean_w = consts.tile([128, 128], BF16, name="negmean_w")
    nc.vector.memset(negmean_w, -1.0 / 256.0)
    sq_w = consts.tile([128, 128], BF16, name="sq_w")
    nc.vector.memset(sq_w, 1.0 / 256.0)
    eps_col = consts.tile([128, 1], F32, name="eps_col")
    nc.vector.memset(eps_col, EPS)

    gamma_sb = consts.tile([128, 2], F32, name="gamma_sb")
    nc.sync.dma_start(out=gamma_sb, in_=gamma.rearrange("(t p) -> p t", p=128))
    beta_sb = consts.tile([128, 2], F32, name="beta_sb")
    nc.sync.dma_start(out=beta_sb, in_=beta.rearrange("(t p) -> p t", p=128))

    # q/k biases: per-partition scalars in the qkT layout ([d on partitions])
    b_in_sb = consts.tile([128, A, 4], F32, name="b_in_sb")
    for a in range(A):
        nc.sync.dma_start(
            out=b_in_sb[:, a, :], in_=b_in[a, 0:512].rearrange("(t p) -> p t", p=128)
        )
    # v bias: a K=1 matmul row (d is a free dim for v), bf16
    b_v_bf = consts.tile([1, A, 256], BF16, name="b_v_bf")
    for a in range(A):
        nc.gpsimd.dma_start(out=b_v_bf[0:1, a, :], in_=b_in[a, 512:768][None, :])
    b_out_sb = consts.tile([128, A, 2], F32, name="b_out_sb")
    for a in range(A):
        nc.sync.dma_start(
            out=b_out_sb[:, a, :], in_=b_out[a].rearrange("(t p) -> p t", p=128)
        )

    # ---- transposed weights, bf16, all areas resident ----
    wt_in = consts.tile([128, 2, A, 768], BF16, name="wt_in")  # [c-in-chunk, cc, a, d]
    wt_out = consts.tile([128, 2, A, 256], BF16, name="wt_out")
    wload = tc.alloc_tile_pool(name="wload", bufs=2)
    wpsum = tc.alloc_tile_pool(name="wpsum", bufs=4, space="PSUM")
    for a in range(A):
        w_raw = wload.tile([128, 6, 256], F32, tag="wraw", name="w_raw")
        nc.sync.dma_start(out=w_raw, in_=W_in[a].rearrange("(t p) c -> p t c", p=128))
        w_bf = wload.tile([128, 6, 256], BF16, tag="wbf", name="w_bf")
        nc.vector.tensor_copy(w_bf, w_raw)
        for t in range(6):
            for cc in range(2):
                tps = wpsum.tile([128, 128], BF16, tag="tps", name="tps")
                nc.tensor.transpose(tps, w_bf[:, t, cc * 128 : (cc + 1) * 128], ident)
                nc.vector.tensor_copy(wt_in[:, cc, a, t * 128 : (t + 1) * 128], tps)
        wo_raw = wload.tile([128, 2, 256], F32, tag="woraw", name="wo_raw")
        nc.sync.dma_start(
            out=wo_raw, in_=W_out[a].rearrange("(t p) c -> p t c", p=128)
        )
        wo_bf = wload.tile([128, 2, 256], BF16, tag="wobf", name="wo_bf")
        nc.vector.tensor_copy(wo_bf, wo_raw)
        for t in range(2):
            for cc in range(2):
                tps = wpsum.tile([128, 128], BF16, tag="tps", name="tps")
                nc.tensor.transpose(tps, wo_bf[:, t, cc * 128 : (cc + 1) * 128], ident)
                nc.vector.tensor_copy(wt_out[:, cc, a, t * 128 : (t + 1) * 128], tps)
    wpsum.release()
    wload.release()

    # ---- input: load x once, cast to compact per-area bf16 [c, l] tiles ----
    xa = consts.tile([128, 2, A, 1024], BF16, name="xa")
    xload = tc.alloc_tile_pool(name="xload", bufs=1)
    xf = xload.tile([128, 2, HDIM, WDIM], F32, name="xf")
    nc.sync.dma_start(out=xf, in_=x.rearrange("(u p) h w -> p u h w", p=128))
    for a in range(A):
        ai, aj = a // 2, a % 2
        for cc in range(2):
            for lh in range(2):
                nc.vector.tensor_copy(
                    xa[:, cc, a, lh * 512 : (lh + 1) * 512].rearrange(
                        "p (r q) -> p r q", r=16
                    ),
                    xf[
                        :,
                        cc,
                        32 * ai + 16 * lh : 32 * ai + 16 * lh + 16,
                        32 * aj : 32 * aj + 32,
                    ],
                )
    xload.release()

    qkp = tc.alloc_tile_pool(name="qkp", bufs=2)
    vp = tc.alloc_tile_pool(name="vp", bufs=2)
    ptp = tc.alloc_tile_pool(name="ptp", bufs=9)
    osbp = tc.alloc_tile_pool(name="osbp", bufs=2)
    ybfp = tc.alloc_tile_pool(name="ybfp", bufs=2)
    statp = tc.alloc_tile_pool(name="statp", bufs=2)
    outp = tc.alloc_tile_pool(name="outp", bufs=2)
    spsum = tc.alloc_tile_pool(name="spsum", bufs=1, space="PSUM")
    wkpsum = tc.alloc_tile_pool(name="wkpsum", bufs=4, space="PSUM")

    out_r = out_ext.rearrange("(u p) h w -> p u h w", p=128)

    for a in range(A):
        ai, aj = a // 2, a % 2

        # ---- qkT projection: [d on partitions, l free], d-chunks q0 q1 k0 k1
        qk = qkp.tile([128, 4, 1024], BF16, tag="qk", name="qk")
        for dt in range(4):
            for lh in range(2):
                ps = wkpsum.tile([128, 512], F32, tag="wk", name="ps_qk")
                for cc in range(2):
                    nc.tensor.matmul(
                        ps,
                        lhsT=wt_in[:, cc, a, dt * 128 : (dt + 1) * 128],
                        rhs=xa[:, cc, a, lh * 512 : (lh + 1) * 512],
                        start=(cc == 0),
                        stop=(cc == 1),
                    )
                nc.vector.tensor_scalar(
                    qk[:, dt, lh * 512 : (lh + 1) * 512],
                    ps,
                    b_in_sb[:, a, dt : dt + 1],
                    None,
                    op0=AluOp.add,
                )

        # ---- v projection: natural [m on partitions, d free]
        v = vp.tile([128, 8, 256], BF16, tag="v", name="v")
        for mc in range(8):
            ps = wkpsum.tile([128, 256], F32, tag="wk", name="ps_v")
            for cc in range(2):
                nc.tensor.matmul(
                    ps,
                    lhsT=xa[:, cc, a, mc * 128 : (mc + 1) * 128],
                    rhs=wt_in[:, cc, a, 512:768],
                    start=(cc == 0),
                    stop=False,
                )
            nc.tensor.matmul(
                ps, lhsT=onesrow, rhs=b_v_bf[0:1, a, :], start=False, stop=True
            )
            nc.vector.tensor_copy(v[:, mc, :], ps)

        o_sb = osbp.tile([128, 2, 1024], BF16, tag="osb", name="o_sb")
        for hg in range(2):
            qt = qk[:, hg, :]
            kt = qk[:, 2 + hg, :]

            # ---- scores S^T = k.T q per head, 4 heads row-tiled; exp on ACT
            pts = []
            for mc in range(8):
                pm = ptp.tile([128, 4, 1024], BF16, tag="pt", name="pm")
                for lhh in range(2):
                    sp = spsum.tile([128, 4, 512], F32, tag="sp", name="sp")
                    for h in range(4):
                        nc.tensor.matmul(
                            sp[:, h, :],
                            lhsT=kt[32 * h : 32 * h + 32, mc * 128 : (mc + 1) * 128],
                            rhs=qt[32 * h : 32 * h + 32, lhh * 512 : (lhh + 1) * 512],
                            start=True,
                            stop=True,
                            tile_position=(32 * h, 0),
                        )
                    nc.scalar.activation(
                        pm[:, :, lhh * 512 : (lhh + 1) * 512],
                        sp,
                        ActFn.Exp,
                        scale=SCALE,
                    )
                pts.append(pm)

            # ---- o^T = P V (col-tiled, 4 heads packed) + denominators
            for lh in range(2):
                ops_ = wkpsum.tile([128, 512], F32, tag="wk", name="ops")
                dps = wkpsum.tile([128, 512], F32, tag="wk", name="dps")
                for mc in range(8):
                    for h in range(4):
                        nc.tensor.matmul(
                            ops_[32 * h : 32 * h + 32, :],
                            lhsT=v[:, mc, hg * 128 + 32 * h : hg * 128 + 32 * h + 32],
                            rhs=pts[mc][:, h, lh * 512 : (lh + 1) * 512],
                            start=(mc == 0),
                            stop=(mc == 7),
                            skip_group_check=True,
                            tile_position=(0, 32 * h),
                        )
                    for h in range(4):
                        nc.tensor.matmul(
                            dps[32 * h : 32 * h + 32, :],
                            lhsT=ones32,
                            rhs=pts[mc][:, h, lh * 512 : (lh + 1) * 512],
                            start=(mc == 0),
                            stop=(mc == 7),
                            skip_group_check=True,
                            tile_position=(0, 32 * h),
                        )
                rd = statp.tile([128, 512], F32, tag="rd", name="rd")
                nc.vector.reciprocal_approx_fast(rd, dps)
                nc.vector.tensor_mul(o_sb[:, hg, lh * 512 : (lh + 1) * 512], ops_, rd)

        # ---- out-projection + residual (+bias) -> y bf16
        ybf = ybfp.tile([128, 2, 1024], BF16, tag="ybf", name="ybf")
        for ec in range(2):
            for lh in range(2):
                ps = wkpsum.tile([128, 512], F32, tag="wk", name="ps_o")
                for cc in range(2):
                    nc.tensor.matmul(
                        ps,
                        lhsT=wt_out[:, cc, a, ec * 128 : (ec + 1) * 128],
                        rhs=o_sb[:, cc, lh * 512 : (lh + 1) * 512],
                        start=(cc == 0),
                        stop=False,
                    )
                nc.tensor.matmul(
                    ps,
                    lhsT=ident,
                    rhs=xa[:, ec, a, lh * 512 : (lh + 1) * 512],
                    start=False,
                    stop=True,
                )
                nc.vector.tensor_scalar(
                    ybf[:, ec, lh * 512 : (lh + 1) * 512],
                    ps,
                    b_out_sb[:, a, ec : ec + 1],
                    None,
                    op0=AluOp.add,
                )

        # ---- LayerNorm over c (stats via ones-matmuls, broadcast over parts)
        outf = outp.tile([128, 2, 1024], F32, tag="outf", name="outf")
        for lh in range(2):
            mps = wkpsum.tile([128, 512], F32, tag="wk", name="mps")
            for cc in range(2):
                nc.tensor.matmul(
                    mps,
                    lhsT=negmean_w,
                    rhs=ybf[:, cc, lh * 512 : (lh + 1) * 512],
                    start=(cc == 0),
                    stop=(cc == 1),
                )
            qps = wkpsum.tile([128, 512], F32, tag="wk", name="qps")
            for cc in range(2):
                ysq = statp.tile([128, 512], BF16, tag="ysq", name="ysq")
                nc.vector.tensor_mul(
                    ysq,
                    ybf[:, cc, lh * 512 : (lh + 1) * 512],
                    ybf[:, cc, lh * 512 : (lh + 1) * 512],
                )
                nc.tensor.matmul(qps, lhsT=sq_w, rhs=ysq, start=(cc == 0), stop=(cc == 1))
            nm = statp.tile([128, 512], BF16, tag="nm", name="nm")
            nc.vector.tensor_copy(nm, mps)
            mu2 = statp.tile([128, 512], BF16, tag="mu2", name="mu2")
            nc.vector.tensor_mul(mu2, nm, nm)
            ve = statp.tile([128, 512], F32, tag="ve", name="ve")
            nc.vector.tensor_sub(ve, qps, mu2)
            lnv = statp.tile([128, 512], F32, tag="lnv", name="lnv")
            nc.scalar.activation(lnv, ve, ActFn.Ln, bias=eps_col, scale=1.0)
            rstd = statp.tile([128, 512], BF16, tag="rstd", name="rstd")
            nc.scalar.activation(rstd, lnv, ActFn.Exp, scale=-0.5)
            for cc in range(2):
                t1 = statp.tile([128, 512], BF16, tag="t1", name="t1")
                nc.vector.tensor_add(t1, ybf[:, cc, lh * 512 : (lh + 1) * 512], nm)
                t2 = statp.tile([128, 512], BF16, tag="t2", name="t2")
                nc.vector.tensor_mul(t2, t1, rstd)
                nc.vector.tensor_scalar(
                    outf[:, cc, lh * 512 : (lh + 1) * 512],
                    t2,
                    gamma_sb[:, cc : cc + 1],
                    beta_sb[:, cc : cc + 1],
                    op0=AluOp.mult,
                    op1=AluOp.add,
                )

        for cc in range(2):
            nc.sync.dma_start(
                out=out_r[:, cc, 32 * ai : 32 * ai + 32, 32 * aj : 32 * aj + 32],
                in_=outf[:, cc, :].rearrange("p (r q) -> p r q", r=32),
            )

    for p in (wkpsum, spsum, outp, statp, ybfp, osbp, ptp, vp, qkp):
        p.release()
    consts.release()


def build_nc():
    nc = bacc.Bacc()
    x = nc.declare_dram_parameter("x", [C, HDIM, WDIM], F32, isOutput=False)
    W_in_t = nc.declare_dram_parameter("W_in", [A, 3 * C, C], F32, isOutput=False)
    b_in_t = nc.declare_dram_parameter("b_in", [A, 3 * C], F32, isOutput=False)
    W_out_t = nc.declare_dram_parameter("W_out", [A, C, C], F32, isOutput=False)
    b_out_t = nc.declare_dram_parameter("b_out", [A, C], F32, isOutput=False)
    gamma_t = nc.declare_dram_parameter("gamma", [C], F32, isOutput=False)
    beta_t = nc.declare_dram_parameter("beta", [C], F32, isOutput=False)
    out_t = nc.declare_dram_parameter("out", [C, HDIM, WDIM], F32, isOutput=True)
    with tile.TileContext(nc) as tc:
        _build_body(
            tc,
            nc,
            x[:],
            W_in_t[:],
            b_in_t[:],
            W_out_t[:],
            b_out_t[:],
            gamma_t[:],
            beta_t[:],
            out_t[:],
        )
    nc.finalize()
    return nc


_NC = None


def _get_nc():
    global _NC
    if _NC is None:
        _NC = build_nc()
    return _NC


def run(inputs, trace=False):
    f32 = lambda t: np.ascontiguousarray(np.asarray(t, dtype=np.float32))
    x = f32(inputs["x"])
    shared = {
        "W_in": f32(inputs["W_in"]),
        "b_in": f32(inputs["b_in"]),
        "W_out": f32(inputs["W_out"]),
        "b_out": f32(inputs["b_out"]),
        "gamma": f32(inputs["gamma"]),
        "beta": f32(inputs["beta"]),
    }
    in_maps = [dict(shared, x=x[b]) for b in range(B)]
    nc = _get_nc()
    res = run_bass_kernel_spmd(nc, in_maps, core_ids=list(range(B)), trace=trace)
    out = np.stack([np.asarray(res.results[b]["out"]) for b in range(B)], axis=0)
    return out.astype(np.float32), res


def kernel(**inputs) -> np.ndarray:
    out, _ = run(inputs, trace=False)
    return out


# revision 20
# speedup vs baseline: 1.1332x; 1.1332x over previous
"""Trainium2 Bass kernel for nn_A2Module (area attention + LayerNorm).

Sharding: data-parallel over batch B=8 across the 8 NeuronCores (one image
per core, weights replicated, no collectives).

Per-core math (shapes per area a of 4, L=1024 tokens, C=256, 8 heads x 32):
  xaT[c,l]   : area tokens, channels on partitions
  qkT[d,l]   = W_in[a][qk] @ xaT  (+b)      d on partitions -> heads are
                                            32-partition slices
  v[m,d]     = xaT.T @ W_in[a][v].T (+b)    natural layout, m on partitions
  S^T[m,l]   = kT_h.T @ qT_h                K=32 row-tiled, 4 heads packed
  P^T        = exp(S^T/sqrt(dh))            ScalarE, PSUM->SBUF bf16
  o^T[d,l]   = sum_m v[m,d] P^T[m,l]        M=32 col-tiled, 4 heads packed
  den[l]     = sum_m P^T[m,l]               ones-matmul, result broadcast
  o^T       /= den                           (softmax normalization)
  y^T        = W_out[a] @ o^T + b + xaT     residual via identity matmul
  out        = LN(y) over c                 stats via ones-matmuls (broadcast
                                            across partitions), applied on DVE
"""

import sys

for _p in ("/opt/trn_rl_repo",):
    if _p not in sys.path:
        sys.path.insert(0, _p)

import numpy as np

import concourse.bacc as bacc
import concourse.bass as bass
import concourse.mybir as mybir
import concourse.tile as tile
from concourse.bass_utils import run_bass_kernel_spmd
from concourse.masks import make_identity

F32 = mybir.dt.float32
BF16 = mybir.dt.bfloat16
AluOp = mybir.AluOpType
ActFn = mybir.ActivationFunctionType

B = 8
C = 256
HDIM = 64
WDIM = 64
A = 4
NH = 8
DH = 32
L = 1024
EPS = 1e-5
SCALE = float(DH) ** -0.5


def _build_body(tc, nc, x, W_in, b_in, W_out, b_out, gamma, beta, out_ext):
    consts = tc.alloc_tile_pool(name="consts", bufs=1)

    ident = consts.tile([128, 128], BF16, name="ident")
    make_identity(nc, ident)
    ones32 = consts.tile([128, 32], BF16, name="ones32")
    nc.vector.memset(ones32, 1.0)
    onesrow = consts.tile([1, 128], BF16, name="onesrow")
    nc.vector.memset(onesrow, 1.0)
    negmean_w = consts.tile([128, 128], BF16, name="negmean_w")
    nc.vector.memset(negmean_w, -1.0 / 256.0)
    sq_w = consts.tile([128, 128], BF16, name="sq_w")
    nc.vector.memset(sq_w, 1.0 / 256.0)
    eps_col = consts.tile([128, 1], F32, name="eps_col")
    nc.vector.memset(eps_col, EPS)

    gamma_sb = consts.tile([128, 2], F32, name="gamma_sb")
    nc.sync.dma_start(out=gamma_sb, in_=gamma.rearrange("(t p) -> p t", p=128))
    beta_sb = consts.tile([128, 2], F32, name="beta_sb")
    nc.sync.dma_start(out=beta_sb, in_=beta.rearrange("(t p) -> p t", p=128))

    # q/k biases: per-partition scalars in the qkT layout ([d on partitions])
    b_in_sb = consts.tile([128, A, 4], F32, name="b_in_sb")
    for a in range(A):
        nc.sync.dma_start(
            out=b_in_sb[:, a, :], in_=b_in[a, 0:512].rearrange("(t p) -> p t", p=128)
        )
    # v bias: a K=1 matmul row (d is a free dim for v), bf16
    b_v_bf = consts.tile([1, A, 256], BF16, name="b_v_bf")
    for a in range(A):
        nc.gpsimd.dma_start(out=b_v_bf[0:1, a, :], in_=b_in[a, 512:768][None, :])
    b_out_sb = consts.tile([128, A, 2], F32, name="b_out_sb")
    for a in range(A):
        nc.sync.dma_start(
            out=b_out_sb[:, a, :], in_=b_out[a].rearrange("(t p) -> p t", p=128)
        )

    # ---- transposed weights, bf16, all areas resident ----
    wt_in = consts.tile([128, 2, A, 768], BF16, name="wt_in")  # [c-in-chunk, cc, a, d]
    wt_out = consts.tile([128, 2, A, 256], BF16, name="wt_out")
    wload = tc.alloc_tile_pool(name="wload", bufs=2)
    wpsum = tc.alloc_tile_pool(name="wpsum", bufs=4, space="PSUM")
    for a in range(A):
        w_raw = wload.tile([128, 6, 256], F32, tag="wraw", name="w_raw")
        nc.sync.dma_start(out=w_raw, in_=W_in[a].rearrange("(t p) c -> p t c", p=128))
        w_bf = wload.tile([128, 6, 256], BF16, tag="wbf", name="w_bf")
        nc.vector.tensor_copy(w_bf, w_raw)
        for t in range(6):
            for cc in range(2):
                tps = wpsum.tile([128, 128], BF16, tag="tps", name="tps")
                nc.tensor.transpose(tps, w_bf[:, t, cc * 128 : (cc + 1) * 128], ident)
                nc.vector.tensor_copy(wt_in[:, cc, a, t * 128 : (t + 1) * 128], tps)
        wo_raw = wload.tile([128, 2, 256], F32, tag="woraw", name="wo_raw")
        nc.sync.dma_start(
            out=wo_raw, in_=W_out[a].rearrange("(t p) c -> p t c", p=128)
        )
        wo_bf = wload.tile([128, 2, 256], BF16, tag="wobf", name="wo_bf")
        nc.vector.tensor_copy(wo_bf, wo_raw)
        for t in range(2):
            for cc in range(2):
                tps = wpsum.tile([128, 128], BF16, tag="tps", name="tps")
                nc.tensor.transpose(tps, wo_bf[:, t, cc * 128 : (cc + 1) * 128], ident)
                nc.vector.tensor_copy(wt_out[:, cc, a, t * 128 : (t + 1) * 128], tps)
    wpsum.release()
    wload.release()

    # ---- input: load x once, cast to compact per-area bf16 [c, l] tiles ----
    xa = consts.tile([128, 2, A, 1024], BF16, name="xa")
    xload = tc.alloc_tile_pool(name="xload", bufs=1)
    xf = xload.tile([128, 2, HDIM, WDIM], F32, name="xf")
    nc.sync.dma_start(out=xf, in_=x.rearrange("(u p) h w -> p u h w", p=128))
    for a in range(A):
        ai, aj = a // 2, a % 2
        for cc in range(2):
            for lh in range(2):
                nc.vector.tensor_copy(
                    xa[:, cc, a, lh * 512 : (lh + 1) * 512].rearrange(
                        "p (r q) -> p r q", r=16
                    ),
                    xf[
                        :,
                        cc,
                        32 * ai + 16 * lh : 32 * ai + 16 * lh + 16,
                        32 * aj : 32 * aj + 32,
                    ],
                )
    xload.release()

    qkp = tc.alloc_tile_pool(name="qkp", bufs=2)
    vp = tc.alloc_tile_pool(name="vp", bufs=2)
    ptp = tc.alloc_tile_pool(name="ptp", bufs=8)
    osbp = tc.alloc_tile_pool(name="osbp", bufs=2)
    ybfp = tc.alloc_tile_pool(name="ybfp", bufs=4)
    statp = tc.alloc_tile_pool(name="statp", bufs=2)
    outp = tc.alloc_tile_pool(name="outp", bufs=2)
    spsum = tc.alloc_tile_pool(name="spsum", bufs=2, space="PSUM")
    wkpsum = tc.alloc_tile_pool(name="wkpsum", bufs=4, space="PSUM")

    out_r = out_ext.rearrange("(u p) h w -> p u h w", p=128)

    nms = [[None, None] for _ in range(A)]
    ves = [[None, None] for _ in range(A)]
    ybfs = [None] * A

    for a in range(A):
        ai, aj = a // 2, a % 2

        # ---- qkT projection: [d on partitions, l free], d-chunks q0 q1 k0 k1
        qk = qkp.tile([128, 4, 1024], BF16, tag="qk", name="qk")
        for dt in range(4):
            for lh in range(2):
                ps = wkpsum.tile([128, 512], F32, tag="wk", name="ps_qk")
                for cc in range(2):
                    nc.tensor.matmul(
                        ps,
                        lhsT=wt_in[:, cc, a, dt * 128 : (dt + 1) * 128],
                        rhs=xa[:, cc, a, lh * 512 : (lh + 1) * 512],
                        start=(cc == 0),
                        stop=(cc == 1),
                    )
                nc.vector.tensor_scalar(
                    qk[:, dt, lh * 512 : (lh + 1) * 512],
                    ps,
                    b_in_sb[:, a, dt : dt + 1],
                    None,
                    op0=AluOp.add,
                )

        # ---- v projection: natural [m on partitions, d free]
        v = vp.tile([128, 8, 256], BF16, tag="v", name="v")
        for mc in range(8):
            ps = wkpsum.tile([128, 256], F32, tag="wk", name="ps_v")
            for cc in range(2):
                nc.tensor.matmul(
                    ps,
                    lhsT=xa[:, cc, a, mc * 128 : (mc + 1) * 128],
                    rhs=wt_in[:, cc, a, 512:768],
                    start=(cc == 0),
                    stop=False,
                )
            nc.tensor.matmul(
                ps, lhsT=onesrow, rhs=b_v_bf[0:1, a, :], start=False, stop=True
            )
            nc.vector.tensor_copy(v[:, mc, :], ps)

        o_sb = osbp.tile([128, 2, 1024], BF16, tag="osb", name="o_sb")
        for hg in range(2):
            qt = qk[:, hg, :]
            kt = qk[:, 2 + hg, :]

            # ---- scores S^T = k.T q per head, 4 heads row-tiled; exp on ACT
            pts = []
            for mc in range(8):
                pm = ptp.tile([128, 4, 1024], BF16, tag="pt", name="pm")
                for lhh in range(2):
                    for hp in range(2):
                        sp = spsum.tile([128, 2, 512], F32, tag="sp", name="sp")
                        for hh in range(2):
                            h = 2 * hp + hh
                            nc.tensor.matmul(
                                sp[:, hh, :],
                                lhsT=kt[
                                    32 * h : 32 * h + 32, mc * 128 : (mc + 1) * 128
                                ],
                                rhs=qt[32 * h : 32 * h + 32, lhh * 512 : (lhh + 1) * 512],
                                start=True,
                                stop=True,
                                tile_position=(32 * h, 0),
                            )
                        nc.scalar.activation(
                            pm[:, 2 * hp : 2 * hp + 2, lhh * 512 : (lhh + 1) * 512],
                            sp,
                            ActFn.Exp,
                            scale=SCALE,
                        )
                pts.append(pm)

            # ---- o^T = P V (col-tiled, 4 heads packed) + denominators
            for lh in range(2):
                ops_ = wkpsum.tile([128, 512], F32, tag="wk", name="ops")
                dps = wkpsum.tile([128, 512], F32, tag="wk", name="dps")
                for mc in range(8):
                    for h in range(4):
                        nc.tensor.matmul(
                            ops_[32 * h : 32 * h + 32, :],
                            lhsT=v[:, mc, hg * 128 + 32 * h : hg * 128 + 32 * h + 32],
                            rhs=pts[mc][:, h, lh * 512 : (lh + 1) * 512],
                            start=(mc == 0),
                            stop=(mc == 7),
                            skip_group_check=True,
                            tile_position=(0, 32 * h),
                        )
                    for h in range(4):
                        nc.tensor.matmul(
                            dps[32 * h : 32 * h + 32, :],
                            lhsT=ones32,
                            rhs=pts[mc][:, h, lh * 512 : (lh + 1) * 512],
                            start=(mc == 0),
                            stop=(mc == 7),
                            skip_group_check=True,
                            tile_position=(0, 32 * h),
                        )
                rd = statp.tile([128, 512], F32, tag="rd", name="rd")
                nc.vector.reciprocal_approx_fast(rd, dps)
                nc.vector.tensor_mul(o_sb[:, hg, lh * 512 : (lh + 1) * 512], ops_, rd)

        # ---- out-projection + residual (+bias) -> y bf16
        ybf = ybfp.tile([128, 2, 1024], BF16, tag="ybf", name="ybf")
        for ec in range(2):
            for lh in range(2):
                ps = wkpsum.tile([128, 512], F32, tag="wk", name="ps_o")
                for cc in range(2):
                    nc.tensor.matmul(
                        ps,
                        lhsT=wt_out[:, cc, a, ec * 128 : (ec + 1) * 128],
                        rhs=o_sb[:, cc, lh * 512 : (lh + 1) * 512],
                        start=(cc == 0),
                        stop=False,
                    )
                nc.tensor.matmul(
                    ps,
                    lhsT=ident,
                    rhs=xa[:, ec, a, lh * 512 : (lh + 1) * 512],
                    start=False,
                    stop=True,
                )
                nc.vector.tensor_scalar(
                    ybf[:, ec, lh * 512 : (lh + 1) * 512],
                    ps,
                    b_out_sb[:, a, ec : ec + 1],
                    None,
                    op0=AluOp.add,
                )

        # ---- LayerNorm stats (ones-matmuls, broadcast across partitions).
        # The Ln/Exp rstd + apply run in one batch at the end of the kernel
        # so the ACT function-table set never thrashes between exp and ln.
        for lh in range(2):
            mps = wkpsum.tile([128, 512], F32, tag="wk", name="mps")
            for cc in range(2):
                nc.tensor.matmul(
                    mps,
                    lhsT=negmean_w,
                    rhs=ybf[:, cc, lh * 512 : (lh + 1) * 512],
                    start=(cc == 0),
                    stop=(cc == 1),
                )
            qps = wkpsum.tile([128, 512], F32, tag="wk", name="qps")
            for cc in range(2):
                ysq = statp.tile([128, 512], BF16, tag="ysq", name="ysq")
                nc.vector.tensor_mul(
                    ysq,
                    ybf[:, cc, lh * 512 : (lh + 1) * 512],
                    ybf[:, cc, lh * 512 : (lh + 1) * 512],
                )
                nc.tensor.matmul(qps, lhsT=sq_w, rhs=ysq, start=(cc == 0), stop=(cc == 1))
            nm = statp.tile([128, 512], BF16, tag="nm", name="nm", bufs=8)
            nc.vector.tensor_copy(nm, mps)
            mu2 = statp.tile([128, 512], BF16, tag="mu2", name="mu2")
            nc.vector.tensor_mul(mu2, nm, nm)
            ve = statp.tile([128, 512], F32, tag="ve", name="ve", bufs=8)
            nc.vector.tensor_sub(ve, qps, mu2)
            nms[a][lh] = nm
            ves[a][lh] = ve
        ybfs[a] = ybf

    # ---- deferred LayerNorm rstd + apply + output DMA ----
    for a in range(A):
        ai, aj = a // 2, a % 2
        outf = outp.tile([128, 2, 1024], F32, tag="outf", name="outf")
        for lh in range(2):
            lnv = statp.tile([128, 512], F32, tag="lnv", name="lnv")
            nc.scalar.activation(lnv, ves[a][lh], ActFn.Ln, bias=eps_col, scale=1.0)
            rstd = statp.tile([128, 512], BF16, tag="rstd", name="rstd")
            nc.scalar.activation(rstd, lnv, ActFn.Exp, scale=-0.5)
            for cc in range(2):
                t1 = statp.tile([128, 512], BF16, tag="t1", name="t1")
                nc.vector.tensor_add(
                    t1, ybfs[a][:, cc, lh * 512 : (lh + 1) * 512], nms[a][lh]
                )
                t2 = statp.tile([128, 512], BF16, tag="t2", name="t2")
                nc.vector.tensor_mul(t2, t1, rstd)
                nc.vector.tensor_scalar(
                    outf[:, cc, lh * 512 : (lh + 1) * 512],
                    t2,
                    gamma_sb[:, cc : cc + 1],
                    beta_sb[:, cc : cc + 1],
                    op0=AluOp.mult,
                    op1=AluOp.add,
                )

        for cc in range(2):
            nc.sync.dma_start(
                out=out_r[:, cc, 32 * ai : 32 * ai + 32, 32 * aj : 32 * aj + 32],
                in_=outf[:, cc, :].rearrange("p (r q) -> p r q", r=32),
            )

    for p in (wkpsum, spsum, outp, statp, ybfp, osbp, ptp, vp, qkp):
        p.release()
    consts.release()


def build_nc():
    nc = bacc.Bacc()
    x = nc.declare_dram_parameter("x", [C, HDIM, WDIM], F32, isOutput=False)
    W_in_t = nc.declare_dram_parameter("W_in", [A, 3 * C, C], F32, isOutput=False)
    b_in_t = nc.declare_dram_parameter("b_in", [A, 3 * C], F32, isOutput=False)
    W_out_t = nc.declare_dram_parameter("W_out", [A, C, C], F32, isOutput=False)
    b_out_t = nc.declare_dram_parameter("b_out", [A, C], F32, isOutput=False)
    gamma_t = nc.declare_dram_parameter("gamma", [C], F32, isOutput=False)
    beta_t = nc.declare_dram_parameter("beta", [C], F32, isOutput=False)
    out_t = nc.declare_dram_parameter("out", [C, HDIM, WDIM], F32, isOutput=True)
    with tile.TileContext(nc) as tc:
        _build_body(
            tc,
            nc,
            x[:],
            W_in_t[:],
            b_in_t[:],
            W_out_t[:],
            b_out_t[:],
            gamma_t[:],
            beta_t[:],
            out_t[:],
        )
    nc.finalize()
    return nc


_NC = None


def _get_nc():
    global _NC
    if _NC is None:
        _NC = build_nc()
    return _NC


def run(inputs, trace=False):
    f32 = lambda t: np.ascontiguousarray(np.asarray(t, dtype=np.float32))
    x = f32(inputs["x"])
    shared = {
        "W_in": f32(inputs["W_in"]),
        "b_in": f32(inputs["b_in"]),
        "W_out": f32(inputs["W_out"]),
        "b_out": f32(inputs["b_out"]),
        "gamma": f32(inputs["gamma"]),
        "beta": f32(inputs["beta"]),
    }
    in_maps = [dict(shared, x=x[b]) for b in range(B)]
    nc = _get_nc()
    res = run_bass_kernel_spmd(nc, in_maps, core_ids=list(range(B)), trace=trace)
    out = np.stack([np.asarray(res.results[b]["out"]) for b in range(B)], axis=0)
    return out.astype(np.float32), res


def kernel(**inputs) -> np.ndarray:
    out, _ = run(inputs, trace=False)
    return out


# revision 23
# speedup vs baseline: 1.2285x; 1.0840x over previous
"""Trainium2 Bass kernel for nn_A2Module (area attention + LayerNorm).

Sharding: data-parallel over batch B=8 across the 8 NeuronCores (one image
per core, weights replicated, no collectives).

Per-core math (shapes per area a of 4, L=1024 tokens, C=256, 8 heads x 32):
  xaT[c,l]   : area tokens, channels on partitions
  qkT[d,l]   = W_in[a][qk] @ xaT  (+b)      d on partitions -> heads are
                                            32-partition slices
  v[m,d]     = xaT.T @ W_in[a][v].T (+b)    natural layout, m on partitions
  S^T[m,l]   = kT_h.T @ qT_h                K=32 row-tiled, 4 heads packed
  P^T        = exp(S^T/sqrt(dh))            ScalarE, PSUM->SBUF bf16
  o^T[d,l]   = sum_m v[m,d] P^T[m,l]        M=32 col-tiled, 4 heads packed
  den[l]     = sum_m P^T[m,l]               ones-matmul, result broadcast
  o^T       /= den                           (softmax normalization)
  y^T        = W_out[a] @ o^T + b + xaT     residual via identity matmul
  out        = LN(y) over c                 stats via ones-matmuls (broadcast
                                            across partitions), applied on DVE
"""

import sys

for _p in ("/opt/trn_rl_repo",):
    if _p not in sys.path:
        sys.path.insert(0, _p)

import numpy as np

import concourse.bacc as bacc
import concourse.bass as bass
import concourse.mybir as mybir
import concourse.tile as tile
from concourse.bass_utils import run_bass_kernel_spmd
from concourse.masks import make_identity

F32 = mybir.dt.float32
BF16 = mybir.dt.bfloat16
AluOp = mybir.AluOpType
ActFn = mybir.ActivationFunctionType

B = 8
C = 256
HDIM = 64
WDIM = 64
A = 4
NH = 8
DH = 32
L = 1024
EPS = 1e-5
SCALE = float(DH) ** -0.5


def _force_combined_act_set():
    """This kernel's only ACT transcendentals are Exp and Ln. Left alone,
    the table picker alternates exp_and_others <-> natural_log, paying a
    ~1.3us ACT_TABLE_LOAD on every switch. Blank every set except
    natural_log_exp_and_others (preserving indices, which walrus uses) so
    exactly one table set is ever loaded."""
    if getattr(bacc, "_act_set_patched", False):
        return
    orig = bacc.get_activation_tables

    def patched(arch):
        t = orig(arch)
        if "natural_log_exp_and_others" not in t:
            return t
        keep = t["natural_log_exp_and_others"]
        return {k: (v if k == "natural_log_exp_and_others" else set()) for k, v in t.items()}

    bacc.get_activation_tables = patched
    bacc._act_set_patched = True


def _build_body(tc, nc, x, W_in, b_in, W_out, b_out, gamma, beta, out_ext):
    consts = tc.alloc_tile_pool(name="consts", bufs=1)

    ident = consts.tile([128, 128], BF16, name="ident")
    make_identity(nc, ident)
    ones32 = consts.tile([128, 32], BF16, name="ones32")
    nc.vector.memset(ones32, 1.0)
    onesrow = consts.tile([1, 128], BF16, name="onesrow")
    nc.vector.memset(onesrow, 1.0)
    negmean_w = consts.tile([128, 128], BF16, name="negmean_w")
    nc.vector.memset(negmean_w, -1.0 / 256.0)
    sq_w = consts.tile([128, 128], BF16, name="sq_w")
    nc.vector.memset(sq_w, 1.0 / 256.0)
    eps_col = consts.tile([128, 1], F32, name="eps_col")
    nc.vector.memset(eps_col, EPS)

    gamma_sb = consts.tile([128, 2], F32, name="gamma_sb")
    nc.sync.dma_start(out=gamma_sb, in_=gamma.rearrange("(t p) -> p t", p=128))
    beta_sb = consts.tile([128, 2], F32, name="beta_sb")
    nc.sync.dma_start(out=beta_sb, in_=beta.rearrange("(t p) -> p t", p=128))

    # q/k biases: per-partition scalars in the qkT layout ([d on partitions])
    b_in_sb = consts.tile([128, A, 4], F32, name="b_in_sb")
    for a in range(A):
        nc.sync.dma_start(
            out=b_in_sb[:, a, :], in_=b_in[a, 0:512].rearrange("(t p) -> p t", p=128)
        )
    # v bias: a K=1 matmul row (d is a free dim for v), bf16
    b_v_bf = consts.tile([1, A, 256], BF16, name="b_v_bf")
    for a in range(A):
        nc.gpsimd.dma_start(out=b_v_bf[0:1, a, :], in_=b_in[a, 512:768][None, :])
    b_out_sb = consts.tile([128, A, 2], F32, name="b_out_sb")
    for a in range(A):
        nc.sync.dma_start(
            out=b_out_sb[:, a, :], in_=b_out[a].rearrange("(t p) -> p t", p=128)
        )

    # ---- transposed weights, bf16, all areas resident ----
    wt_in = consts.tile([128, 2, A, 768], BF16, name="wt_in")  # [c-in-chunk, cc, a, d]
    wt_out = consts.tile([128, 2, A, 256], BF16, name="wt_out")
    wload = tc.alloc_tile_pool(name="wload", bufs=2)
    wpsum = tc.alloc_tile_pool(name="wpsum", bufs=4, space="PSUM")
    for a in range(A):
        w_raw = wload.tile([128, 6, 256], F32, tag="wraw", name="w_raw")
        nc.sync.dma_start(out=w_raw, in_=W_in[a].rearrange("(t p) c -> p t c", p=128))
        w_bf = wload.tile([128, 6, 256], BF16, tag="wbf", name="w_bf")
        nc.vector.tensor_copy(w_bf, w_raw)
        for t in range(6):
            for cc in range(2):
                tps = wpsum.tile([128, 128], BF16, tag="tps", name="tps")
                nc.tensor.transpose(tps, w_bf[:, t, cc * 128 : (cc + 1) * 128], ident)
                nc.vector.tensor_copy(wt_in[:, cc, a, t * 128 : (t + 1) * 128], tps)
        wo_raw = wload.tile([128, 2, 256], F32, tag="woraw", name="wo_raw")
        nc.sync.dma_start(
            out=wo_raw, in_=W_out[a].rearrange("(t p) c -> p t c", p=128)
        )
        wo_bf = wload.tile([128, 2, 256], BF16, tag="wobf", name="wo_bf")
        nc.vector.tensor_copy(wo_bf, wo_raw)
        for t in range(2):
            for cc in range(2):
                tps = wpsum.tile([128, 128], BF16, tag="tps", name="tps")
                nc.tensor.transpose(tps, wo_bf[:, t, cc * 128 : (cc + 1) * 128], ident)
                nc.vector.tensor_copy(wt_out[:, cc, a, t * 128 : (t + 1) * 128], tps)
    wpsum.release()
    wload.release()

    # ---- input: load x once, cast to compact per-area bf16 [c, l] tiles ----
    xa = consts.tile([128, 2, A, 1024], BF16, name="xa")
    xload = tc.alloc_tile_pool(name="xload", bufs=1)
    xf = xload.tile([128, 2, HDIM, WDIM], F32, name="xf")
    nc.sync.dma_start(out=xf, in_=x.rearrange("(u p) h w -> p u h w", p=128))
    for a in range(A):
        ai, aj = a // 2, a % 2
        for cc in range(2):
            for lh in range(2):
                nc.vector.tensor_copy(
                    xa[:, cc, a, lh * 512 : (lh + 1) * 512].rearrange(
                        "p (r q) -> p r q", r=16
                    ),
                    xf[
                        :,
                        cc,
                        32 * ai + 16 * lh : 32 * ai + 16 * lh + 16,
                        32 * aj : 32 * aj + 32,
                    ],
                )
    xload.release()

    qkp = tc.alloc_tile_pool(name="qkp", bufs=2)
    vp = tc.alloc_tile_pool(name="vp", bufs=2)
    ptp = tc.alloc_tile_pool(name="ptp", bufs=8)
    osbp = tc.alloc_tile_pool(name="osbp", bufs=2)
    ybfp = tc.alloc_tile_pool(name="ybfp", bufs=4)
    statp = tc.alloc_tile_pool(name="statp", bufs=2)
    outp = tc.alloc_tile_pool(name="outp", bufs=2)
    spsum = tc.alloc_tile_pool(name="spsum", bufs=3, space="PSUM")
    wkpsum = tc.alloc_tile_pool(name="wkpsum", bufs=2, space="PSUM")

    out_r = out_ext.rearrange("(u p) h w -> p u h w", p=128)

    nms = [[None, None] for _ in range(A)]
    ves = [[None, None] for _ in range(A)]
    ybfs = [None] * A

    for a in range(A):
        ai, aj = a // 2, a % 2

        # ---- qkT projection: [d on partitions, l free], d-chunks q0 q1 k0 k1
        qk = qkp.tile([128, 4, 1024], BF16, tag="qk", name="qk")
        for dt in range(4):
            for lh in range(2):
                ps = wkpsum.tile([128, 512], F32, tag="wk", name="ps_qk")
                for cc in range(2):
                    nc.tensor.matmul(
                        ps,
                        lhsT=wt_in[:, cc, a, dt * 128 : (dt + 1) * 128],
                        rhs=xa[:, cc, a, lh * 512 : (lh + 1) * 512],
                        start=(cc == 0),
                        stop=(cc == 1),
                    )
                nc.vector.tensor_scalar(
                    qk[:, dt, lh * 512 : (lh + 1) * 512],
                    ps,
                    b_in_sb[:, a, dt : dt + 1],
                    None,
                    op0=AluOp.add,
                )

        # ---- v projection: natural [m on partitions, d free]
        v = vp.tile([128, 8, 256], BF16, tag="v", name="v")
        for mc in range(8):
            ps = wkpsum.tile([128, 256], F32, tag="wk", name="ps_v")
            for cc in range(2):
                nc.tensor.matmul(
                    ps,
                    lhsT=xa[:, cc, a, mc * 128 : (mc + 1) * 128],
                    rhs=wt_in[:, cc, a, 512:768],
                    start=(cc == 0),
                    stop=False,
                )
            nc.tensor.matmul(
                ps, lhsT=onesrow, rhs=b_v_bf[0:1, a, :], start=False, stop=True
            )
            nc.vector.tensor_copy(v[:, mc, :], ps)

        o_sb = osbp.tile([128, 2, 1024], BF16, tag="osb", name="o_sb")
        for hg in range(2):
            qt = qk[:, hg, :]
            kt = qk[:, 2 + hg, :]

            # ---- scores S^T = k.T q per head, 4 heads row-tiled; exp on ACT
            pts = []
            for mc in range(8):
                pm = ptp.tile([128, 4, 1024], BF16, tag="pt", name="pm")
                for lhh in range(2):
                    for hp in range(2):
                        sp = spsum.tile([128, 2, 512], F32, tag="sp", name="sp")
                        for hh in range(2):
                            h = 2 * hp + hh
                            nc.tensor.matmul(
                                sp[:, hh, :],
                                lhsT=kt[
                                    32 * h : 32 * h + 32, mc * 128 : (mc + 1) * 128
                                ],
                                rhs=qt[32 * h : 32 * h + 32, lhh * 512 : (lhh + 1) * 512],
                                start=True,
                                stop=True,
                                tile_position=(32 * h, 0),
                            )
                        nc.scalar.activation(
                            pm[:, 2 * hp : 2 * hp + 2, lhh * 512 : (lhh + 1) * 512],
                            sp,
                            ActFn.Exp,
                            scale=SCALE,
                        )
                pts.append(pm)

            # ---- o^T = P V (col-tiled, 4 heads packed) + denominators
            for lh in range(2):
                ops_ = wkpsum.tile([128, 512], F32, tag="wk", name="ops")
                dps = wkpsum.tile([128, 512], F32, tag="wk", name="dps")
                for mc in range(8):
                    for h in range(4):
                        nc.tensor.matmul(
                            ops_[32 * h : 32 * h + 32, :],
                            lhsT=v[:, mc, hg * 128 + 32 * h : hg * 128 + 32 * h + 32],
                            rhs=pts[mc][:, h, lh * 512 : (lh + 1) * 512],
                            start=(mc == 0),
                            stop=(mc == 7),
                            skip_group_check=True,
                            tile_position=(0, 32 * h),
                        )
                    for h in range(4):
                        nc.tensor.matmul(
                            dps[32 * h : 32 * h + 32, :],
                            lhsT=ones32,
                            rhs=pts[mc][:, h, lh * 512 : (lh + 1) * 512],
                            start=(mc == 0),
                            stop=(mc == 7),
                            skip_group_check=True,
                            tile_position=(0, 32 * h),
                        )
                rd = statp.tile([128, 512], F32, tag="rd", name="rd")
                nc.vector.reciprocal_approx_fast(rd, dps)
                nc.vector.tensor_mul(o_sb[:, hg, lh * 512 : (lh + 1) * 512], ops_, rd)

        # ---- out-projection + residual (+bias) -> y bf16
        ybf = ybfp.tile([128, 2, 1024], BF16, tag="ybf", name="ybf")
        for ec in range(2):
            for lh in range(2):
                ps = wkpsum.tile([128, 512], F32, tag="wk", name="ps_o")
                for cc in range(2):
                    nc.tensor.matmul(
                        ps,
                        lhsT=wt_out[:, cc, a, ec * 128 : (ec + 1) * 128],
                        rhs=o_sb[:, cc, lh * 512 : (lh + 1) * 512],
                        start=(cc == 0),
                        stop=False,
                    )
                nc.tensor.matmul(
                    ps,
                    lhsT=ident,
                    rhs=xa[:, ec, a, lh * 512 : (lh + 1) * 512],
                    start=False,
                    stop=True,
                )
                nc.vector.tensor_scalar(
                    ybf[:, ec, lh * 512 : (lh + 1) * 512],
                    ps,
                    b_out_sb[:, a, ec : ec + 1],
                    None,
                    op0=AluOp.add,
                )

        # ---- LayerNorm stats (ones-matmuls, broadcast across partitions).
        # The Ln/Exp rstd + apply run in one batch at the end of the kernel
        # so the ACT function-table set never thrashes between exp and ln.
        for lh in range(2):
            mps = wkpsum.tile([128, 512], F32, tag="wk", name="mps")
            for cc in range(2):
                nc.tensor.matmul(
                    mps,
                    lhsT=negmean_w,
                    rhs=ybf[:, cc, lh * 512 : (lh + 1) * 512],
                    start=(cc == 0),
                    stop=(cc == 1),
                )
            qps = wkpsum.tile([128, 512], F32, tag="wk", name="qps")
            for cc in range(2):
                ysq = statp.tile([128, 512], BF16, tag="ysq", name="ysq")
                nc.vector.tensor_mul(
                    ysq,
                    ybf[:, cc, lh * 512 : (lh + 1) * 512],
                    ybf[:, cc, lh * 512 : (lh + 1) * 512],
                )
                nc.tensor.matmul(qps, lhsT=sq_w, rhs=ysq, start=(cc == 0), stop=(cc == 1))
            nm = statp.tile([128, 512], BF16, tag="nm", name="nm", bufs=8)
            nc.vector.tensor_copy(nm, mps)
            mu2 = statp.tile([128, 512], BF16, tag="mu2", name="mu2")
            nc.vector.tensor_mul(mu2, nm, nm)
            ve = statp.tile([128, 512], F32, tag="ve", name="ve", bufs=8)
            nc.vector.tensor_sub(ve, qps, mu2)
            nms[a][lh] = nm
            ves[a][lh] = ve
        ybfs[a] = ybf

    # ---- deferred LayerNorm rstd + apply + output DMA ----
    for a in range(A):
        ai, aj = a // 2, a % 2
        outf = outp.tile([128, 2, 1024], F32, tag="outf", name="outf")
        for lh in range(2):
            lnv = statp.tile([128, 512], F32, tag="lnv", name="lnv")
            nc.scalar.activation(lnv, ves[a][lh], ActFn.Ln, bias=eps_col, scale=1.0)
            rstd = statp.tile([128, 512], BF16, tag="rstd", name="rstd")
            nc.scalar.activation(rstd, lnv, ActFn.Exp, scale=-0.5)
            for cc in range(2):
                t1 = statp.tile([128, 512], BF16, tag="t1", name="t1")
                nc.vector.tensor_add(
                    t1, ybfs[a][:, cc, lh * 512 : (lh + 1) * 512], nms[a][lh]
                )
                t2 = statp.tile([128, 512], BF16, tag="t2", name="t2")
                nc.vector.tensor_mul(t2, t1, rstd)
                nc.vector.tensor_scalar(
                    outf[:, cc, lh * 512 : (lh + 1) * 512],
                    t2,
                    gamma_sb[:, cc : cc + 1],
                    beta_sb[:, cc : cc + 1],
                    op0=AluOp.mult,
                    op1=AluOp.add,
                )

        for cc in range(2):
            nc.sync.dma_start(
                out=out_r[:, cc, 32 * ai : 32 * ai + 32, 32 * aj : 32 * aj + 32],
                in_=outf[:, cc, :].rearrange("p (r q) -> p r q", r=32),
            )

    for p in (wkpsum, spsum, outp, statp, ybfp, osbp, ptp, vp, qkp):
        p.release()
    consts.release()


def build_nc():
    _force_combined_act_set()
    nc = bacc.Bacc()
    x = nc.declare_dram_parameter("x", [C, HDIM, WDIM], F32, isOutput=False)
    W_in_t = nc.declare_dram_parameter("W_in", [A, 3 * C, C], F32, isOutput=False)
    b_in_t = nc.declare_dram_parameter("b_in", [A, 3 * C], F32, isOutput=False)
    W_out_t = nc.declare_dram_parameter("W_out", [A, C, C], F32, isOutput=False)
    b_out_t = nc.declare_dram_parameter("b_out", [A, C], F32, isOutput=False)
    gamma_t = nc.declare_dram_parameter("gamma", [C], F32, isOutput=False)
    beta_t = nc.declare_dram_parameter("beta", [C], F32, isOutput=False)
    out_t = nc.declare_dram_parameter("out", [C, HDIM, WDIM], F32, isOutput=True)
    with tile.TileContext(nc) as tc:
        _build_body(
            tc,
            nc,
            x[:],
            W_in_t[:],
            b_in_t[:],
            W_out_t[:],
            b_out_t[:],
            gamma_t[:],
            beta_t[:],
            out_t[:],
        )
    nc.finalize()
    return nc


_NC = None


def _get_nc():
    global _NC
    if _NC is None:
        _NC = build_nc()
    return _NC


def run(inputs, trace=False):
    f32 = lambda t: np.ascontiguousarray(np.asarray(t, dtype=np.float32))
    x = f32(inputs["x"])
    shared = {
        "W_in": f32(inputs["W_in"]),
        "b_in": f32(inputs["b_in"]),
        "W_out": f32(inputs["W_out"]),
        "b_out": f32(inputs["b_out"]),
        "gamma": f32(inputs["gamma"]),
        "beta": f32(inputs["beta"]),
    }
    in_maps = [dict(shared, x=x[b]) for b in range(B)]
    nc = _get_nc()
    res = run_bass_kernel_spmd(nc, in_maps, core_ids=list(range(B)), trace=trace)
    out = np.stack([np.asarray(res.results[b]["out"]) for b in range(B)], axis=0)
    return out.astype(np.float32), res


def kernel(**inputs) -> np.ndarray:
    out, _ = run(inputs, trace=False)
    return out
